# revision 6
# baseline (speedup 1.0000x reference)
"""Trainium2 Bass kernel for EquidistantDiscreteContinuousConv3d.

Math: out = conv3d(x, einsum('ogk,kzyx->ogzyx', weight, psi_local), stride 2,
pad 2) + bias, with x [2,8,128,128,128] -> out [2,16,64,64,64].

Key structural fact: psi_local is EXACTLY zero outside the central 3x3x3
block (the radius-2 boundary offsets fall outside `r <= r_cutoff` due to the
+1e-12 inside the sqrt, and hat(1)=0 kills the |d|=2 axis taps). So the
dense 5^3 kernel is a dense 3^3 = 27-tap stencil: 9 (dy,dx) taps, each with
a 3-wide dz band.

Sharding: 8 cores = batch(2) x z-groups(4); core (b,gz) computes
out[b, :, 16gz:16gz+16] from input slab z in [32gz-1, 32gz+32] (33 slices,
zero-padded out of range). No collectives.

Device mapping: tensor engine contracts K = (z_rel(15) x ic(8)) = 120
partitions. M packs (ozr x oc): oz split into 3 chunks of (7,7,2) rows,
each chunk a z-window of (15,15,5) slices at slab offsets (0,14,28) so the
band mapping zr = 2*ozr + dz + 1 is identical for all chunks -> ONE banded
weight tile [120, 9*112] serves every group. 9 (dy,dx) taps accumulate in
PSUM per group; N = 512 = 8 oy x 64 ox, 8 y-groups per chunk -> 24 psum
groups x 9 matmuls = 216 matmuls of 512 cols (vs 312 for the 33-tap band).

Input arrives de-interleaved (ye 65, py 2, px 2, xe 65) per z-slice so each
tap is a contiguous [8,1,1,64] window. Chunks are DMA'd in 8 ye-slabs each
(c0 on SP queue, c1 on ACT queue, c2 whole on SP) so group g only waits for
slabs <= g. Output staged to bf16 (4 rotating stage buffers) and DMA'd per
group; host upcasts and adds bias.

Raw Bacc pipeline per core (static, fully unrolled):
  SP  : 8 c0 ye-slab DMAs + whole c2 DMA, then end-of-run sem clear
  ACT : wtile DMA, 8 c1 ye-slab DMAs, then 24 out DMAs
  PE  : 96 warm-up matmuls, then 24 groups x 9 banded matmuls (bank k%8)
  DVE : 24 psum->stage bf16 copies (stage slot k%4)
"""

import os

import ml_dtypes
import numpy as np

BF16 = ml_dtypes.bfloat16

IC, OC = 8, 16
NZS = 33  # slab z-slices per core
PLANE = 65 * 2 * 2 * 65  # de-interleaved (ye, py, px, xe) plane = 16900
CHUNK_BASE = (0, 14, 28)  # slab z offset of each chunk
CHUNK_NZ = (15, 15, 5)  # z-slices per chunk
CHUNK_OZ0 = (0, 7, 14)  # first oz row of each chunk
CHUNK_NOZ = (7, 7, 2)  # oz rows per chunk
N_CORES = 8
NG = 24  # psum groups: (c0,g),(c1,g) interleaved for g in 0..8, then (c2,g)

_MODULE = None
LAST_RESULT = None  # BassKernelResults of the most recent run (for test harness)


def _groups():
    gs = []
    for g in range(8):
        gs.append((0, g))
        gs.append((1, g))
    for g in range(8):
        gs.append((2, g))
    return gs


def _slab_bounds(i):
    """ye-slab i covers rows [0,9) for i=0 else [8i+1, 8i+9)."""
    lo = 0 if i == 0 else 8 * i + 1
    return lo, 8 * i + 9


def _build_module():
    from contextlib import ExitStack

    import concourse.bacc as bacc
    import concourse.mybir as mybir

    f32 = mybir.dt.float32
    bf16 = mybir.dt.bfloat16

    nc = bacc.Bacc()
    x_in = [
        nc.dram_tensor(f"x{c}", [8 * CHUNK_NZ[c], PLANE], bf16, kind="ExternalInput")
        for c in range(3)
    ]
    w_in = nc.dram_tensor("wc", [120, 9 * 112], bf16, kind="ExternalInput")
    out = nc.dram_tensor("out", [16, 16, 64, 64], bf16, kind="ExternalOutput")

    groups = _groups()

    with ExitStack() as ctx:
        wsem = ctx.enter_context(nc.semaphore("wsem"))
        xs = [ctx.enter_context(nc.semaphore(f"xs{i}")) for i in range(2)]
        xb = [ctx.enter_context(nc.semaphore(f"xb{i}")) for i in range(2)]
        pesem = ctx.enter_context(nc.semaphore("pesem"))
        dvsem = ctx.enter_context(nc.semaphore("dvsem"))
        oss = [ctx.enter_context(nc.semaphore(f"os{i}")) for i in range(2)]
        wtile = ctx.enter_context(nc.sbuf_tensor("wtile", [120, 9 * 112], bf16))
        xts = [
            ctx.enter_context(
                nc.sbuf_tensor(f"xt{c}", [8 * CHUNK_NZ[c], PLANE], bf16)
            )
            for c in range(3)
        ]
        stgs = [
            ctx.enter_context(nc.sbuf_tensor(f"stg{i}", [128, 512], bf16))
            for i in range(4)
        ]
        pss = [
            ctx.enter_context(nc.psum_tensor(f"ps{i}", [128, 512], f32))
            for i in range(8)
        ]
        vs = [
            t[:].rearrange("p (a b c d) -> p a b c d", a=65, b=2, c=2, d=65)
            for t in xts
        ]

        with nc.Block() as block:

            @block.sync
            def _(sp):
                # c0 in 8 ye-slabs, then c2 whole (needed only from group 16).
                # Same-sem DMAs are paced so they never run concurrently: the
                # per-parity counts are completion-exact (<=2 in flight).
                for i in range(8):
                    if i == 1:
                        sp.wait_ge(xs[0], 16)  # let slab0 land at full BW
                    elif i >= 2:
                        sp.wait_ge(xs[i % 2], 16 * (i // 2))
                    lo, hi = _slab_bounds(i)
                    sp.dma_start(
                        out=xts[0][:, lo * 260 : hi * 260],
                        in_=x_in[0][:, lo * 260 : hi * 260],
                    ).then_inc(xs[i % 2], 16)
                sp.wait_ge(xs[0], 16 * 4)
                sp.dma_start(out=xts[2][:], in_=x_in[2][:]).then_inc(xs[0], 16)
                # re-execution safety: clear sems once everything is done
                sp.wait_ge(oss[0], 16 * 12)
                sp.wait_ge(oss[1], 16 * 12)
                for sem in (wsem, xs[0], xs[1], xb[0], xb[1], pesem, dvsem,
                            oss[0], oss[1]):
                    sp.sem_clear(sem)

            @block.scalar
            def _(act):
                act.dma_start(out=wtile[:], in_=w_in[:]).then_inc(wsem, 16)
                for i in range(8):
                    if i == 1:
                        act.wait_ge(xb[0], 16)
                    elif i >= 2:
                        act.wait_ge(xb[i % 2], 16 * (i // 2))
                    lo, hi = _slab_bounds(i)
                    act.dma_start(
                        out=xts[1][:, lo * 260 : hi * 260],
                        in_=x_in[1][:, lo * 260 : hi * 260],
                    ).then_inc(xb[i % 2], 16)
                for k, (c, g) in enumerate(groups):
                    act.wait_ge(dvsem, k + 1)
                    if k >= 2:
                        act.wait_ge(oss[k % 2], 16 * (k // 2))
                    dst = out[
                        CHUNK_OZ0[c] : CHUNK_OZ0[c] + CHUNK_NOZ[c],
                        :,
                        8 * g : 8 * g + 8,
                        :,
                    ].rearrange("a b c d -> (a b) (c d)")
                    act.dma_start(
                        out=dst, in_=stgs[k % 4][0 : 16 * CHUNK_NOZ[c], :]
                    ).then_inc(oss[k % 2], 16)

            @block.tensor
            def _(pe):
                # HAM warm-up: cheap N=64 throwaway matmuls keep PE busy from
                # the preamble until the first input lands; psum bank 7 is
                # discarded by its first start=True.
                for _ in range(96):
                    pe.matmul(
                        pss[7][0:64, 0:64], wtile[:, 0:64], wtile[:, 0:64],
                        start=True, stop=True,
                    )
                pe.wait_ge(wsem, 16)
                for k, (c, g) in enumerate(groups):
                    if c == 0:
                        pe.wait_ge(xs[g % 2], 16 * (g // 2 + 1))
                    elif c == 1:
                        pe.wait_ge(xb[g % 2], 16 * (g // 2 + 1))
                    elif g == 0:
                        pe.wait_ge(xs[0], 16 * 5)  # whole c2 (9th SP DMA)
                    if k >= 8:
                        pe.wait_ge(dvsem, k - 7)  # psum bank k%8 evacuated
                    v = vs[c]
                    kp = 8 * CHUNK_NZ[c]
                    m = 16 * CHUNK_NOZ[c]
                    ps = pss[k % 8]
                    for j in range(9):
                        dy, dx = j // 3 - 1, j % 3 - 1
                        jy, py = divmod(dy + 1, 2)
                        jx, px = divmod(dx + 1, 2)
                        rhs = v[
                            0:kp,
                            8 * g + jy : 8 * g + jy + 8,
                            py : py + 1,
                            px : px + 1,
                            jx : jx + 64,
                        ]
                        mm = pe.matmul(
                            ps[0:m, :],
                            wtile[0:kp, j * 112 : j * 112 + m],
                            rhs,
                            start=(j == 0),
                            stop=(j == 8),
                        )
                        if j == 8:
                            mm.then_inc(pesem, 1)

            @block.vector
            def _(dve):
                for k, (c, g) in enumerate(groups):
                    m = 16 * CHUNK_NOZ[c]
                    dve.wait_ge(pesem, k + 1)
                    if k >= 4:
                        # stage slot k%4 free once out-DMA k-4 (same parity)
                        # completed; same-parity outs are paced, so exact.
                        dve.wait_ge(oss[k % 2], 16 * (k // 2 - 1))
                    dve.tensor_copy(
                        out=stgs[k % 4][0:m, :], in_=pss[k % 8][0:m, :]
                    ).then_inc(dvsem, 1)

    nc.compile()
    return nc


def _get_module():
    global _MODULE
    if _MODULE is None:
        _MODULE = _build_module()
    return _MODULE


def _band_weights(w5):
    """wc[(zr*8+ic), j*112 + ozr*16 + oc] banded weights, zr = 2*ozr+dz+1."""
    w3 = w5[:, :, 1:4, 1:4, 1:4]  # central 3x3x3 (rest is exactly zero)
    wc = np.zeros((120, 9, 7, 16), np.float32)
    for j in range(9):
        dy, dx = j // 3 - 1, j % 3 - 1
        for dz in (-1, 0, 1):
            blk = w3[:, :, dz + 1, dy + 1, dx + 1].T  # [ic, oc]
            for ozr in range(7):
                zr = 2 * ozr + dz + 1
                wc[zr * 8 : (zr + 1) * 8, j, ozr, :] = blk
    return np.ascontiguousarray(wc.reshape(120, 9 * 112))


def _shard_core_input(x, b, gz):
    """Per-core input: 3 z-chunks, de-interleaved planes [nz*8, PLANE]."""
    xp = np.zeros((IC, NZS, 130, 130), np.float32)
    z_lo = 32 * gz - 1
    src_lo, src_hi = max(0, z_lo), min(128, z_lo + NZS)
    xp[:, src_lo - z_lo : src_hi - z_lo, 1:129, 1:129] = x[b, :, src_lo:src_hi]
    # de-interleave: y = 2*ye+py-1, x = 2*xe+px-1
    xd = xp.reshape(IC, NZS, 65, 2, 65, 2).transpose(0, 1, 2, 3, 5, 4)
    chunks = []
    for c in range(3):
        base, nz = CHUNK_BASE[c], CHUNK_NZ[c]
        ch = xd[:, base : base + nz].transpose(1, 0, 2, 3, 4, 5)
        chunks.append(np.ascontiguousarray(ch.reshape(nz * 8, PLANE), dtype=BF16))
    return chunks


def kernel(x, weight, bias, psi_local):
    global LAST_RESULT
    from concourse.bass_utils import run_bass_kernel_spmd

    x = np.asarray(x, np.float32)
    weight = np.asarray(weight, np.float32)
    bias = np.asarray(bias, np.float32)
    psi_local = np.asarray(psi_local, np.float32)

    w5 = np.einsum("ogk,kzyx->ogzyx", weight, psi_local).astype(np.float32)
    wc = _band_weights(w5).astype(BF16)

    in_maps = []
    for core in range(N_CORES):
        b, gz = divmod(core, 4)
        c0, c1, c2 = _shard_core_input(x, b, gz)
        in_maps.append({"x0": c0, "x1": c1, "x2": c2, "wc": wc})

    nc = _get_module()
    trace = bool(int(os.environ.get("KERNEL_TRACE", "0")))
    res = run_bass_kernel_spmd(
        nc, in_maps, core_ids=list(range(N_CORES)), trace=trace
    )
    LAST_RESULT = res

    out = np.empty((2, OC, 64, 64, 64), np.float32)
    for core in range(N_CORES):
        b, gz = divmod(core, 4)
        out[b, :, 16 * gz : 16 * gz + 16] = (
            res.results[core]["out"].astype(np.float32).transpose(1, 0, 2, 3)
        )
    out += bias[None, :, None, None, None]
    return out


# revision 17
# speedup vs baseline: 2.3586x; 2.3586x over previous
"""Trainium2 Bass kernel for EquidistantDiscreteContinuousConv3d.

Math: out = conv3d(x, einsum('ogk,kzyx->ogzyx', weight, psi_local), stride 2,
pad 2) + bias, with x [2,8,128,128,128] -> out [2,16,64,64,64].

Key structural fact: psi_local is EXACTLY zero outside the central 3x3x3
block (the radius-2 boundary offsets fall outside `r <= r_cutoff` due to the
+1e-12 inside the sqrt, and hat(1)=0 kills the |d|=2 axis taps). So the
dense 5^3 kernel is a dense 3^3 = 27-tap stencil: 9 (dy,dx) taps, each with
a 3-wide dz band.

Sharding: 8 cores = batch(2) x z-groups(4); core (b,gz) computes
out[b, :, 16gz:16gz+16] from input slab z in [32gz-1, 32gz+31] (33 slices,
zero-padded out of range). No collectives.

Device mapping: tensor engine contracts K = (z_rel(15) x ic(8)) = 120
partitions. M packs (ozr x oc): oz split into 3 chunks of (7,7,2) rows,
each chunk a z-window of (15,15,5) slices at slab offsets (0,14,28) so the
band mapping zr = 2*ozr + dz + 1 is identical for all chunks -> ONE banded
weight tile serves every group. Weight tap blocks are 128 columns apart
(256B-aligned; a 112-col stride measurably slows LDWEIGHTS/matmul overlap).
9 (dy,dx) taps accumulate in PSUM per group; N = 512 = 8 oy x 64 ox, 8
y-groups per chunk -> 24 psum groups x 9 matmuls = 216 matmuls of 512 cols.

Input arrives de-interleaved (ye 65, py 2, px 2, xe 65) per z-slice so each
tap is a contiguous [8,1,1,64] window. Per-DMA-queue bandwidth is only
~128 GB/s, so traffic is spread over FOUR queues; same-sem DMAs are paced so
per-sem completion counts stay exact (<=2 in flight per queue):
  SP    : c0 ye-slabs 0-5
  ACT   : wtile, c1 ye-slabs 0-7
  DVE   : c2 whole, c0 ye-slabs 6-7, then 24 psum->stage bf16 copies
  GPSIMD: 24 out DMAs (stage slot k%4, parity-paced)
Output is staged to bf16; host upcasts and adds the (zero) bias.
"""

import os

import ml_dtypes
import numpy as np

BF16 = ml_dtypes.bfloat16

IC, OC = 8, 16
NZS = 33  # slab z-slices per core
PLANE = 65 * 2 * 2 * 65  # de-interleaved (ye, py, px, xe) plane = 16900
CHUNK_BASE = (0, 14, 28)  # slab z offset of each chunk
CHUNK_NZ = (15, 15, 5)  # z-slices per chunk
CHUNK_OZ0 = (0, 7, 14)  # first oz row of each chunk
CHUNK_NOZ = (7, 7, 2)  # oz rows per chunk
N_CORES = 8
NG = 24  # psum groups: (c0,g),(c1,g) interleaved for g in 0..8, then (c2,g)

_MODULE = None
LAST_RESULT = None  # BassKernelResults of the most recent run (for test harness)


def _groups():
    gs = []
    for g in range(8):
        gs.append((0, g))
        gs.append((1, g))
    for g in range(8):
        gs.append((2, g))
    return gs


def _slab_bounds(i):
    """ye-slab i covers rows [0,9) for i=0 else [8i+1, 8i+9)."""
    lo = 0 if i == 0 else 8 * i + 1
    return lo, 8 * i + 9


def _build_module():
    from contextlib import ExitStack

    import concourse.bacc as bacc
    import concourse.mybir as mybir

    f32 = mybir.dt.float32
    bf16 = mybir.dt.bfloat16

    nc = bacc.Bacc()
    x_in = [
        nc.dram_tensor(f"x{c}", [8 * CHUNK_NZ[c], PLANE], bf16, kind="ExternalInput")
        for c in range(3)
    ]
    w_in = nc.dram_tensor("wc", [120, 9 * 128], bf16, kind="ExternalInput")
    out = nc.dram_tensor("out", [16, 16, 64, 64], bf16, kind="ExternalOutput")

    groups = _groups()

    with ExitStack() as ctx:
        wsem = ctx.enter_context(nc.semaphore("wsem"))
        xs = [ctx.enter_context(nc.semaphore(f"xs{i}")) for i in range(2)]
        xb = [ctx.enter_context(nc.semaphore(f"xb{i}")) for i in range(2)]
        xcs = [ctx.enter_context(nc.semaphore(f"xc{i}")) for i in range(2)]
        pesem = ctx.enter_context(nc.semaphore("pesem"))
        dvsem = ctx.enter_context(nc.semaphore("dvsem"))
        oss = [ctx.enter_context(nc.semaphore(f"os{i}")) for i in range(2)]
        wtile = ctx.enter_context(nc.sbuf_tensor("wtile", [120, 9 * 128], bf16))
        xts = [
            ctx.enter_context(
                nc.sbuf_tensor(f"xt{c}", [8 * CHUNK_NZ[c], PLANE], bf16)
            )
            for c in range(3)
        ]
        stgs = [
            ctx.enter_context(nc.sbuf_tensor(f"stg{i}", [128, 512], bf16))
            for i in range(4)
        ]
        pss = [
            ctx.enter_context(nc.psum_tensor(f"ps{i}", [128, 512], f32))
            for i in range(8)
        ]
        vs = [
            t[:].rearrange("p (a b c d) -> p a b c d", a=65, b=2, c=2, d=65)
            for t in xts
        ]

        with nc.Block() as block:

            @block.sync
            def _(sp):
                # c0 ye-slabs 0-7, then c2 ye-slabs 0-7 (c2 is only needed
                # from group 16 on). Same-sem DMAs are paced so they never
                # run concurrently: per-parity completion counts are exact
                # (<=2 in flight).
                for i in range(8):
                    if i == 1:
                        sp.wait_ge(xs[0], 16)  # let slab0 land at full BW
                    elif i >= 2:
                        sp.wait_ge(xs[i % 2], 16 * (i // 2))
                    lo, hi = _slab_bounds(i)
                    sp.dma_start(
                        out=xts[0][:, lo * 260 : hi * 260],
                        in_=x_in[0][:, lo * 260 : hi * 260],
                    ).then_inc(xs[i % 2], 16)
                for i in range(8):
                    if i < 2:
                        sp.wait_ge(xs[i % 2], 16 * 4)  # c0 slab 6/7 done
                    else:
                        sp.wait_ge(xcs[i % 2], 16 * (i // 2))
                    lo, hi = _slab_bounds(i)
                    sp.dma_start(
                        out=xts[2][:, lo * 260 : hi * 260],
                        in_=x_in[2][:, lo * 260 : hi * 260],
                    ).then_inc(xcs[i % 2], 16)
                # re-execution safety: clear sems once everything is done
                sp.wait_ge(oss[0], 16 * 12)
                sp.wait_ge(oss[1], 16 * 12)
                for sem in (wsem, xs[0], xs[1], xb[0], xb[1], xcs[0], xcs[1],
                            pesem, dvsem, oss[0], oss[1]):
                    sp.sem_clear(sem)

            @block.scalar
            def _(act):
                act.dma_start(out=wtile[:], in_=w_in[:]).then_inc(wsem, 16)
                for i in range(8):
                    if i == 1:
                        act.wait_ge(xb[0], 16)
                    elif i >= 2:
                        act.wait_ge(xb[i % 2], 16 * (i // 2))
                    lo, hi = _slab_bounds(i)
                    act.dma_start(
                        out=xts[1][:, lo * 260 : hi * 260],
                        in_=x_in[1][:, lo * 260 : hi * 260],
                    ).then_inc(xb[i % 2], 16)
                for k, (c, g) in enumerate(groups):
                    act.wait_ge(dvsem, k + 1)
                    if k >= 2:
                        act.wait_ge(oss[k % 2], 16 * (k // 2))
                    dst = out[
                        CHUNK_OZ0[c] : CHUNK_OZ0[c] + CHUNK_NOZ[c],
                        :,
                        8 * g : 8 * g + 8,
                        :,
                    ].rearrange("a b c d -> (a b) (c d)")
                    act.dma_start(
                        out=dst, in_=stgs[k % 4][0 : 16 * CHUNK_NOZ[c], :]
                    ).then_inc(oss[k % 2], 16)

            @block.vector
            def _(dve):
                for k, (c, g) in enumerate(groups):
                    m = 16 * CHUNK_NOZ[c]
                    dve.wait_ge(pesem, k + 1)
                    if k >= 4:
                        # stage slot k%4 free once out-DMA k-4 (same parity)
                        # completed; same-parity outs are paced, so exact.
                        dve.wait_ge(oss[k % 2], 16 * (k // 2 - 1))
                    dve.tensor_copy(
                        out=stgs[k % 4][0:m, :], in_=pss[k % 8][0:m, :]
                    ).then_inc(dvsem, 1)

            @block.tensor
            def _(pe):
                # HAM warm-up: cheap N=64 throwaway matmuls keep PE busy from
                # the preamble until the first input lands; psum bank 7 is
                # discarded by its first start=True.
                for _ in range(96):
                    pe.matmul(
                        pss[7][0:64, 0:64], wtile[:, 0:64], wtile[:, 0:64],
                        start=True, stop=True,
                    )
                pe.wait_ge(wsem, 16)
                for k, (c, g) in enumerate(groups):
                    if c == 0:
                        pe.wait_ge(xs[g % 2], 16 * (g // 2 + 1))
                    elif c == 1:
                        pe.wait_ge(xb[g % 2], 16 * (g // 2 + 1))
                    else:
                        pe.wait_ge(xcs[g % 2], 16 * (g // 2 + 1))
                    if k >= 8:
                        pe.wait_ge(dvsem, k - 7)  # psum bank k%8 evacuated
                    v = vs[c]
                    kp = 8 * CHUNK_NZ[c]
                    m = 16 * CHUNK_NOZ[c]
                    ps = pss[k % 8]
                    for j in range(9):
                        dy, dx = j // 3 - 1, j % 3 - 1
                        jy, py = divmod(dy + 1, 2)
                        jx, px = divmod(dx + 1, 2)
                        rhs = v[
                            0:kp,
                            8 * g + jy : 8 * g + jy + 8,
                            py : py + 1,
                            px : px + 1,
                            jx : jx + 64,
                        ]
                        mm = pe.matmul(
                            ps[0:m, :],
                            wtile[0:kp, j * 128 : j * 128 + m],
                            rhs,
                            start=(j == 0),
                            stop=(j == 8),
                        )
                        if j == 8:
                            mm.then_inc(pesem, 1)

    nc.compile()
    return nc


def _get_module():
    global _MODULE
    if _MODULE is None:
        _MODULE = _build_module()
    return _MODULE


def _band_weights(w5):
    """wc[(zr*8+ic), j*128 + ozr*16 + oc] banded weights, zr = 2*ozr+dz+1."""
    w3 = w5[:, :, 1:4, 1:4, 1:4]  # central 3x3x3 (rest is exactly zero)
    wc = np.zeros((120, 9, 128), np.float32)
    for j in range(9):
        dy, dx = j // 3 - 1, j % 3 - 1
        for dz in (-1, 0, 1):
            blk = w3[:, :, dz + 1, dy + 1, dx + 1].T  # [ic, oc]
            for ozr in range(7):
                zr = 2 * ozr + dz + 1
                wc[zr * 8 : (zr + 1) * 8, j, ozr * 16 : ozr * 16 + 16] = blk
    return np.ascontiguousarray(wc.reshape(120, 9 * 128))


def _shard_core_input(x, b, gz):
    """Per-core input: 3 z-chunks, de-interleaved planes [nz*8, PLANE]."""
    xp = np.zeros((IC, NZS, 130, 130), np.float32)
    z_lo = 32 * gz - 1
    src_lo, src_hi = max(0, z_lo), min(128, z_lo + NZS)
    xp[:, src_lo - z_lo : src_hi - z_lo, 1:129, 1:129] = x[b, :, src_lo:src_hi]
    # de-interleave: y = 2*ye+py-1, x = 2*xe+px-1
    xd = xp.reshape(IC, NZS, 65, 2, 65, 2).transpose(0, 1, 2, 3, 5, 4)
    chunks = []
    for c in range(3):
        base, nz = CHUNK_BASE[c], CHUNK_NZ[c]
        ch = xd[:, base : base + nz].transpose(1, 0, 2, 3, 4, 5)
        chunks.append(np.ascontiguousarray(ch.reshape(nz * 8, PLANE), dtype=BF16))
    return chunks


def kernel(x, weight, bias, psi_local):
    global LAST_RESULT
    from concourse.bass_utils import run_bass_kernel_spmd

    x = np.asarray(x, np.float32)
    weight = np.asarray(weight, np.float32)
    bias = np.asarray(bias, np.float32)
    psi_local = np.asarray(psi_local, np.float32)

    w5 = np.einsum("ogk,kzyx->ogzyx", weight, psi_local).astype(np.float32)
    wc = _band_weights(w5).astype(BF16)

    in_maps = []
    for core in range(N_CORES):
        b, gz = divmod(core, 4)
        c0, c1, c2 = _shard_core_input(x, b, gz)
        in_maps.append({"x0": c0, "x1": c1, "x2": c2, "wc": wc})

    nc = _get_module()
    trace = bool(int(os.environ.get("KERNEL_TRACE", "0")))
    res = run_bass_kernel_spmd(
        nc, in_maps, core_ids=list(range(N_CORES)), trace=trace
    )
    LAST_RESULT = res

    out = np.empty((2, OC, 64, 64, 64), np.float32)
    for core in range(N_CORES):
        b, gz = divmod(core, 4)
        out[b, :, 16 * gz : 16 * gz + 16] = (
            res.results[core]["out"].astype(np.float32).transpose(1, 0, 2, 3)
        )
    out += bias[None, :, None, None, None]
    return out


# revision 23
# speedup vs baseline: 2.9703x; 1.2593x over previous
"""Trainium2 Bass kernel for EquidistantDiscreteContinuousConv3d.

Math: out = conv3d(x, einsum('ogk,kzyx->ogzyx', weight, psi_local), stride 2,
pad 2) + bias, with x [2,8,128,128,128] -> out [2,16,64,64,64].

Key structural fact: psi_local is EXACTLY zero outside the central 3x3x3
block (the radius-2 boundary offsets fall outside `r <= r_cutoff` due to the
+1e-12 inside the sqrt, and hat(1)=0 kills the |d|=2 axis taps). So the
dense 5^3 kernel is a dense 3^3 = 27-tap stencil: 9 (dy,dx) taps, each with
a 3-wide dz band.

Sharding: 8 cores = batch(2) x z-groups(4); core (b,gz) computes
out[b, :, 16gz:16gz+16] from input slab z in [32gz-1, 32gz+31] (33 slices,
zero-padded out of range). No collectives.

Device mapping: tensor engine contracts K = (z_rel(15) x ic(8)) = 120
partitions. M packs (ozr x oc): oz split into 3 chunks of (7,7,2) rows,
each chunk a z-window of (15,15,5) slices at slab offsets (0,14,28) so the
band mapping zr = 2*ozr + dz + 1 is identical for all chunks -> ONE banded
weight tile serves every group. Weight tap blocks are 128 columns apart
(256B-aligned; a 112-col stride measurably slows LDWEIGHTS/matmul overlap).
9 (dy,dx) taps accumulate in PSUM per group; N = 512 = 8 oy x 64 ox, 8
y-groups per chunk -> 24 psum groups x 9 matmuls = 216 matmuls of 512 cols.

Input arrives de-interleaved (ye 65, py 2, px 2, xe 65) per z-slice so each
tap is a contiguous [8,1,1,64] window. Per-DMA-queue bandwidth is only
~128 GB/s, so traffic is spread over FOUR queues; same-sem DMAs are paced so
per-sem completion counts stay exact (<=2 in flight per queue):
  SP    : c0 ye-slabs 0-5
  ACT   : wtile, c1 ye-slabs 0-7
  DVE   : c2 whole, c0 ye-slabs 6-7, then 24 psum->stage bf16 copies
  GPSIMD: 24 out DMAs (stage slot k%4, parity-paced)
Output is staged to bf16; host upcasts and adds the (zero) bias.
"""

import os

import ml_dtypes
import numpy as np

BF16 = ml_dtypes.bfloat16

IC, OC = 8, 16
NZS = 33  # slab z-slices per core
PLANE = 65 * 2 * 2 * 65  # de-interleaved (ye, py, px, xe) plane = 16900
CHUNK_BASE = (0, 14, 28)  # slab z offset of each chunk
CHUNK_NZ = (15, 15, 5)  # z-slices per chunk
CHUNK_OZ0 = (0, 7, 14)  # first oz row of each chunk
CHUNK_NOZ = (7, 7, 2)  # oz rows per chunk
N_CORES = 8
NG = 24  # psum groups: (c0,g),(c1,g) interleaved for g in 0..8, then (c2,g)

_MODULE = None
LAST_RESULT = None  # BassKernelResults of the most recent run (for test harness)


def _groups():
    gs = []
    for g in range(8):
        gs.append((0, g))
        gs.append((1, g))
    for g in range(8):
        gs.append((2, g))
    return gs


def _slab_bounds(i):
    """ye-slab i covers rows [0,9) for i=0 else [8i+1, 8i+9)."""
    lo = 0 if i == 0 else 8 * i + 1
    return lo, 8 * i + 9


def _build_module():
    from contextlib import ExitStack

    import concourse.bacc as bacc
    import concourse.mybir as mybir

    f32 = mybir.dt.float32
    bf16 = mybir.dt.bfloat16

    nc = bacc.Bacc()
    x_in = [
        nc.dram_tensor(f"x{c}", [8 * CHUNK_NZ[c], PLANE], bf16, kind="ExternalInput")
        for c in range(3)
    ]
    w_in = nc.dram_tensor("wc", [120, 9 * 128], bf16, kind="ExternalInput")
    # group-major contiguous staging: rows O_k..O_k+m_k are group k's
    # (ozr*16+oc) rows, columns (oy_rel*64+ox); host reassembles.
    out = nc.dram_tensor("outf", [2048, 512], bf16, kind="ExternalOutput")

    groups = _groups()
    offs = []
    o = 0
    for c, g in groups:
        offs.append(o)
        o += 16 * CHUNK_NOZ[c]

    with ExitStack() as ctx:
        wsem = ctx.enter_context(nc.semaphore("wsem"))
        xs = [ctx.enter_context(nc.semaphore(f"xs{i}")) for i in range(2)]
        xb = [ctx.enter_context(nc.semaphore(f"xb{i}")) for i in range(2)]
        xcs = [ctx.enter_context(nc.semaphore(f"xc{i}")) for i in range(2)]
        pesem = ctx.enter_context(nc.semaphore("pesem"))
        dvsem = ctx.enter_context(nc.semaphore("dvsem"))
        oss = [ctx.enter_context(nc.semaphore(f"os{i}")) for i in range(2)]
        wtile = ctx.enter_context(nc.sbuf_tensor("wtile", [120, 9 * 128], bf16))
        xts = [
            ctx.enter_context(
                nc.sbuf_tensor(f"xt{c}", [8 * CHUNK_NZ[c], PLANE], bf16)
            )
            for c in range(3)
        ]
        stgs = [
            ctx.enter_context(nc.sbuf_tensor(f"stg{i}", [128, 512], bf16))
            for i in range(4)
        ]
        pss = [
            ctx.enter_context(nc.psum_tensor(f"ps{i}", [128, 512], f32))
            for i in range(8)
        ]
        vs = [
            t[:].rearrange("p (a b c d) -> p a b c d", a=65, b=2, c=2, d=65)
            for t in xts
        ]

        with nc.Block() as block:

            @block.sync
            def _(sp):
                # c0 ye-slabs 0-7, then c2 ye-slabs 0-7 (c2 is only needed
                # from group 16 on). Same-sem DMAs are paced so they never
                # run concurrently: per-parity completion counts are exact
                # (<=2 in flight).
                for i in range(8):
                    if i == 1:
                        sp.wait_ge(xs[0], 16)  # let slab0 land at full BW
                    elif i >= 2:
                        sp.wait_ge(xs[i % 2], 16 * (i // 2))
                    lo, hi = _slab_bounds(i)
                    sp.dma_start(
                        out=xts[0][:, lo * 260 : hi * 260],
                        in_=x_in[0][:, lo * 260 : hi * 260],
                    ).then_inc(xs[i % 2], 16)
                for i in range(8):
                    if i < 2:
                        sp.wait_ge(xs[i % 2], 16 * 4)  # c0 slab 6/7 done
                    else:
                        sp.wait_ge(xcs[i % 2], 16 * (i // 2))
                    lo, hi = _slab_bounds(i)
                    sp.dma_start(
                        out=xts[2][:, lo * 260 : hi * 260],
                        in_=x_in[2][:, lo * 260 : hi * 260],
                    ).then_inc(xcs[i % 2], 16)

            @block.scalar
            def _(act):
                act.dma_start(out=wtile[:], in_=w_in[:]).then_inc(wsem, 16)
                for i in range(8):
                    if i == 1:
                        act.wait_ge(xb[0], 16)
                    elif i >= 2:
                        act.wait_ge(xb[i % 2], 16 * (i // 2))
                    lo, hi = _slab_bounds(i)
                    act.dma_start(
                        out=xts[1][:, lo * 260 : hi * 260],
                        in_=x_in[1][:, lo * 260 : hi * 260],
                    ).then_inc(xb[i % 2], 16)
            @block.gpsimd
            def _(gp):
                for k, (c, g) in enumerate(groups):
                    m = 16 * CHUNK_NOZ[c]
                    gp.wait_ge(dvsem, k + 1)
                    if k >= 2:
                        gp.wait_ge(oss[k % 2], 16 * (k // 2))
                    gp.dma_start(
                        out=out[offs[k] : offs[k] + m, :],
                        in_=stgs[k % 4][0:m, :],
                    ).then_inc(oss[k % 2], 16)
                # No end-of-run sem_clear: each kernel() call loads a fresh
                # NEFF (sems start at 0, single execution), and waiting on
                # SWDGE-incremented sems from another engine wedges the
                # device (microbench4).

            @block.vector
            def _(dve):
                for k, (c, g) in enumerate(groups):
                    m = 16 * CHUNK_NOZ[c]
                    dve.wait_ge(pesem, k + 1)
                    if k >= 4:
                        # stage slot k%4 free once out-DMA k-4 (same parity)
                        # completed; same-parity outs are paced, so exact.
                        dve.wait_ge(oss[k % 2], 16 * (k // 2 - 1))
                    dve.tensor_copy(
                        out=stgs[k % 4][0:m, :], in_=pss[k % 8][0:m, :]
                    ).then_inc(dvsem, 1)

            @block.tensor
            def _(pe):
                # HAM warm-up: cheap N=64 throwaway matmuls keep PE busy from
                # the preamble until the first input lands; psum bank 7 is
                # discarded by its first start=True.
                for _ in range(96):
                    pe.matmul(
                        pss[7][0:64, 0:64], wtile[:, 0:64], wtile[:, 0:64],
                        start=True, stop=True,
                    )
                pe.wait_ge(wsem, 16)
                for k, (c, g) in enumerate(groups):
                    if c == 0:
                        pe.wait_ge(xs[g % 2], 16 * (g // 2 + 1))
                    elif c == 1:
                        pe.wait_ge(xb[g % 2], 16 * (g // 2 + 1))
                    else:
                        pe.wait_ge(xcs[g % 2], 16 * (g // 2 + 1))
                    if k >= 8:
                        pe.wait_ge(dvsem, k - 7)  # psum bank k%8 evacuated
                    v = vs[c]
                    kp = 8 * CHUNK_NZ[c]
                    m = 16 * CHUNK_NOZ[c]
                    ps = pss[k % 8]
                    for j in range(9):
                        dy, dx = j // 3 - 1, j % 3 - 1
                        jy, py = divmod(dy + 1, 2)
                        jx, px = divmod(dx + 1, 2)
                        rhs = v[
                            0:kp,
                            8 * g + jy : 8 * g + jy + 8,
                            py : py + 1,
                            px : px + 1,
                            jx : jx + 64,
                        ]
                        mm = pe.matmul(
                            ps[0:m, :],
                            wtile[0:kp, j * 128 : j * 128 + m],
                            rhs,
                            start=(j == 0),
                            stop=(j == 8),
                        )
                        if j == 8:
                            mm.then_inc(pesem, 1)

    nc.compile()
    return nc


def _get_module():
    global _MODULE
    if _MODULE is None:
        _MODULE = _build_module()
    return _MODULE


def _band_weights(w5):
    """wc[(zr*8+ic), j*128 + ozr*16 + oc] banded weights, zr = 2*ozr+dz+1."""
    w3 = w5[:, :, 1:4, 1:4, 1:4]  # central 3x3x3 (rest is exactly zero)
    wc = np.zeros((120, 9, 128), np.float32)
    for j in range(9):
        dy, dx = j // 3 - 1, j % 3 - 1
        for dz in (-1, 0, 1):
            blk = w3[:, :, dz + 1, dy + 1, dx + 1].T  # [ic, oc]
            for ozr in range(7):
                zr = 2 * ozr + dz + 1
                wc[zr * 8 : (zr + 1) * 8, j, ozr * 16 : ozr * 16 + 16] = blk
    return np.ascontiguousarray(wc.reshape(120, 9 * 128))


def _shard_core_input(x, b, gz):
    """Per-core input: 3 z-chunks, de-interleaved planes [nz*8, PLANE]."""
    xp = np.zeros((IC, NZS, 130, 130), np.float32)
    z_lo = 32 * gz - 1
    src_lo, src_hi = max(0, z_lo), min(128, z_lo + NZS)
    xp[:, src_lo - z_lo : src_hi - z_lo, 1:129, 1:129] = x[b, :, src_lo:src_hi]
    # de-interleave: y = 2*ye+py-1, x = 2*xe+px-1
    xd = xp.reshape(IC, NZS, 65, 2, 65, 2).transpose(0, 1, 2, 3, 5, 4)
    chunks = []
    for c in range(3):
        base, nz = CHUNK_BASE[c], CHUNK_NZ[c]
        ch = xd[:, base : base + nz].transpose(1, 0, 2, 3, 4, 5)
        chunks.append(np.ascontiguousarray(ch.reshape(nz * 8, PLANE), dtype=BF16))
    return chunks


def kernel(x, weight, bias, psi_local):
    global LAST_RESULT
    from concourse.bass_utils import run_bass_kernel_spmd

    x = np.asarray(x, np.float32)
    weight = np.asarray(weight, np.float32)
    bias = np.asarray(bias, np.float32)
    psi_local = np.asarray(psi_local, np.float32)

    w5 = np.einsum("ogk,kzyx->ogzyx", weight, psi_local).astype(np.float32)
    wc = _band_weights(w5).astype(BF16)

    in_maps = []
    for core in range(N_CORES):
        b, gz = divmod(core, 4)
        c0, c1, c2 = _shard_core_input(x, b, gz)
        in_maps.append({"x0": c0, "x1": c1, "x2": c2, "wc": wc})

    nc = _get_module()
    trace = bool(int(os.environ.get("KERNEL_TRACE", "0")))
    res = run_bass_kernel_spmd(
        nc, in_maps, core_ids=list(range(N_CORES)), trace=trace
    )
    LAST_RESULT = res

    groups = _groups()
    out = np.empty((2, OC, 64, 64, 64), np.float32)
    oc_t = np.empty((16, OC, 64, 64), np.float32)  # [oz, oc, oy, ox] per core
    for core in range(N_CORES):
        b, gz = divmod(core, 4)
        buf = res.results[core]["outf"].astype(np.float32)
        o = 0
        for c, g in groups:
            noz = CHUNK_NOZ[c]
            arr = buf[o : o + 16 * noz].reshape(noz, 16, 8, 64)
            o += 16 * noz
            oc_t[CHUNK_OZ0[c] : CHUNK_OZ0[c] + noz, :, 8 * g : 8 * g + 8] = arr
        out[b, :, 16 * gz : 16 * gz + 16] = oc_t.transpose(1, 0, 2, 3)
    out += bias[None, :, None, None, None]
    return out


# revision 30
# speedup vs baseline: 3.2022x; 1.0780x over previous
"""Trainium2 Bass kernel for EquidistantDiscreteContinuousConv3d.

Math: out = conv3d(x, einsum('ogk,kzyx->ogzyx', weight, psi_local), stride 2,
pad 2) + bias, with x [2,8,128,128,128] -> out [2,16,64,64,64].

Key structural fact: psi_local is EXACTLY zero outside the central 3x3x3
block (the radius-2 boundary offsets fall outside `r <= r_cutoff` due to the
+1e-12 inside the sqrt, and hat(1)=0 kills the |d|=2 axis taps). So the
dense 5^3 kernel is a dense 3^3 = 27-tap stencil: 9 (dy,dx) taps, each with
a 3-wide dz band.

Sharding: 8 cores = batch(2) x z-groups(4); core (b,gz) computes
out[b, :, 16gz:16gz+16] from input slab z in [32gz-1, 32gz+31] (33 slices,
zero-padded out of range). No collectives.

Device mapping: tensor engine contracts K = (z_rel(15) x ic(8)) = 120
partitions. M packs (ozr x oc): oz split into 3 chunks of (7,7,2) rows,
each chunk a z-window of (15,15,5) slices at slab offsets (0,14,28) so the
band mapping zr = 2*ozr + dz + 1 is identical for all chunks -> ONE banded
weight tile serves every group. Weight tap blocks are 128 columns apart
(256B-aligned; a 112-col stride measurably slows LDWEIGHTS/matmul overlap).
9 (dy,dx) taps accumulate in PSUM per group; N = 512 = 8 oy x 64 ox, 8
y-groups per chunk -> 24 psum groups x 9 matmuls = 216 matmuls of 512 cols.

Input arrives de-interleaved (ye 65, py 2, px 2, xe 65) per z-slice so each
tap is a contiguous [8,1,1,64] window. Per-DMA-queue bandwidth is only
~128 GB/s, so traffic is spread over FOUR queues; same-sem DMAs are paced so
per-sem completion counts stay exact (<=2 in flight per queue):
  SP    : c0 ye-slabs 0-5
  ACT   : wtile, c1 ye-slabs 0-7
  DVE   : c2 whole, c0 ye-slabs 6-7, then 24 psum->stage bf16 copies
  GPSIMD: 24 out DMAs (stage slot k%4, parity-paced)
Output is staged to bf16; host upcasts and adds the (zero) bias.
"""

import os

import ml_dtypes
import numpy as np

BF16 = ml_dtypes.bfloat16

IC, OC = 8, 16
NZS = 33  # slab z-slices per core
PLANE = 65 * 2 * 2 * 65  # de-interleaved (ye, py, px, xe) plane = 16900
CHUNK_BASE = (0, 14, 28)  # slab z offset of each chunk
CHUNK_NZ = (15, 15, 5)  # z-slices per chunk
CHUNK_OZ0 = (0, 7, 14)  # first oz row of each chunk
CHUNK_NOZ = (7, 7, 2)  # oz rows per chunk
N_CORES = 8
NG = 24  # psum groups: (c0,g),(c1,g) interleaved for g in 0..8, then (c2,g)

_MODULE = None
LAST_RESULT = None  # BassKernelResults of the most recent run (for test harness)


def _groups():
    gs = []
    for g in range(8):
        gs.append((0, g))
        gs.append((1, g))
    for g in range(8):
        gs.append((2, g))
    return gs


def _slab_bounds(i):
    """ye-slab i covers rows [0,9) for i=0 else [8i+1, 8i+9)."""
    lo = 0 if i == 0 else 8 * i + 1
    return lo, 8 * i + 9


def _build_module():
    from contextlib import ExitStack

    import concourse.bacc as bacc
    import concourse.mybir as mybir

    f32 = mybir.dt.float32
    bf16 = mybir.dt.bfloat16

    nc = bacc.Bacc()
    x_in = [
        nc.dram_tensor(f"x{c}", [8 * CHUNK_NZ[c], PLANE], bf16, kind="ExternalInput")
        for c in range(3)
    ]
    w_in = nc.dram_tensor("wc", [120, 9 * 128], bf16, kind="ExternalInput")
    # group-major contiguous staging: rows O_k..O_k+m_k are group k's
    # (ozr*16+oc) rows, columns (oy_rel*64+ox); host reassembles.
    out = nc.dram_tensor("outf", [2048, 512], bf16, kind="ExternalOutput")

    groups = _groups()
    offs = []
    o = 0
    for c, g in groups:
        offs.append(o)
        o += 16 * CHUNK_NOZ[c]

    with ExitStack() as ctx:
        wsem = ctx.enter_context(nc.semaphore("wsem"))
        xs = [ctx.enter_context(nc.semaphore(f"xs{i}")) for i in range(2)]
        xb = [ctx.enter_context(nc.semaphore(f"xb{i}")) for i in range(2)]
        xcs = [ctx.enter_context(nc.semaphore(f"xc{i}")) for i in range(2)]
        pesem = ctx.enter_context(nc.semaphore("pesem"))
        dvsem = ctx.enter_context(nc.semaphore("dvsem"))
        oss = [ctx.enter_context(nc.semaphore(f"os{i}")) for i in range(2)]
        wtile = ctx.enter_context(nc.sbuf_tensor("wtile", [120, 9 * 128], bf16))
        # xt2 is padded to 120 partitions: rows 40-119 are memset to zero so
        # c2 matmuls can run full-tile (K=120) — small-tile matmuls stream at
        # less than half rate. wtile rows 40-119 are zero in the ozr<2
        # columns, so the padding contributes exactly 0.
        msem = ctx.enter_context(nc.semaphore("msem"))
        xts = [
            ctx.enter_context(nc.sbuf_tensor(f"xt{c}", [120, PLANE], bf16))
            for c in range(3)
        ]
        stgs = [
            ctx.enter_context(nc.sbuf_tensor(f"stg{i}", [128, 512], bf16))
            for i in range(4)
        ]
        pss = [
            ctx.enter_context(nc.psum_tensor(f"ps{i}", [128, 512], f32))
            for i in range(8)
        ]
        vs = [
            t[:].rearrange("p (a b c d) -> p a b c d", a=65, b=2, c=2, d=65)
            for t in xts
        ]

        with nc.Block() as block:

            @block.sync
            def _(sp):
                # c0 ye-slabs 0-7, then c2 ye-slabs 0-7 (c2 is only needed
                # from group 16 on). Same-sem DMAs are paced so they never
                # run concurrently: per-parity completion counts are exact
                # (<=2 in flight).
                for i in range(8):
                    if i == 1:
                        sp.wait_ge(xs[0], 16)  # let slab0 land at full BW
                    elif i >= 2:
                        sp.wait_ge(xs[i % 2], 16 * (i // 2))
                    lo, hi = _slab_bounds(i)
                    sp.dma_start(
                        out=xts[0][:, lo * 260 : hi * 260],
                        in_=x_in[0][:, lo * 260 : hi * 260],
                    ).then_inc(xs[i % 2], 16)
                for i in range(8):
                    if i == 0:
                        sp.wait_ge(msem, 1)  # xt2 fully zeroed first
                    if i < 2:
                        sp.wait_ge(xs[i % 2], 16 * 4)  # c0 slab 6/7 done
                    else:
                        sp.wait_ge(xcs[i % 2], 16 * (i // 2))
                    lo, hi = _slab_bounds(i)
                    sp.dma_start(
                        out=xts[2][0:40, lo * 260 : hi * 260],
                        in_=x_in[2][:, lo * 260 : hi * 260],
                    ).then_inc(xcs[i % 2], 16)

            @block.scalar
            def _(act):
                act.dma_start(out=wtile[:], in_=w_in[:]).then_inc(wsem, 16)
                for i in range(8):
                    if i == 1:
                        act.wait_ge(xb[0], 16)
                    elif i >= 2:
                        act.wait_ge(xb[i % 2], 16 * (i // 2))
                    lo, hi = _slab_bounds(i)
                    act.dma_start(
                        out=xts[1][:, lo * 260 : hi * 260],
                        in_=x_in[1][:, lo * 260 : hi * 260],
                    ).then_inc(xb[i % 2], 16)
            @block.gpsimd
            def _(gp):
                # zero ALL of xt2 (a partition-offset memset fails codegen);
                # the c2 slab DMAs overwrite rows 0-39 afterwards (SP gates
                # on msem). Rows 40-119 stay zero for the K=120 pad.
                gp.memset(xts[2][:], 0).then_inc(msem, 1)
                for k, (c, g) in enumerate(groups):
                    m = 16 * CHUNK_NOZ[c]
                    gp.wait_ge(dvsem, k + 1)
                    if k >= 2:
                        gp.wait_ge(oss[k % 2], 16 * (k // 2))
                    gp.dma_start(
                        out=out[offs[k] : offs[k] + m, :],
                        in_=stgs[k % 4][0:m, :],
                    ).then_inc(oss[k % 2], 16)
                # No end-of-run sem_clear: each kernel() call loads a fresh
                # NEFF (sems start at 0, single execution), and waiting on
                # SWDGE-incremented sems from another engine wedges the
                # device (microbench4).

            @block.vector
            def _(dve):
                for k, (c, g) in enumerate(groups):
                    m = 16 * CHUNK_NOZ[c]
                    dve.wait_ge(pesem, k + 1)
                    if k >= 4:
                        # stage slot k%4 free once out-DMA k-4 (same parity)
                        # completed; same-parity outs are paced, so exact.
                        dve.wait_ge(oss[k % 2], 16 * (k // 2 - 1))
                    dve.tensor_copy(
                        out=stgs[k % 4][0:m, :], in_=pss[k % 8][0:m, :]
                    ).then_inc(dvsem, 1)

            @block.tensor
            def _(pe):
                # HAM warm-up: cheap N=64 throwaway matmuls keep PE busy from
                # the preamble until the first input lands; psum bank 7 is
                # discarded by its first start=True.
                for _ in range(96):
                    pe.matmul(
                        pss[7][0:64, 0:64], wtile[:, 0:64], wtile[:, 0:64],
                        start=True, stop=True,
                    )
                pe.wait_ge(wsem, 16)
                for k, (c, g) in enumerate(groups):
                    if c == 0:
                        pe.wait_ge(xs[g % 2], 16 * (g // 2 + 1))
                    elif c == 1:
                        pe.wait_ge(xb[g % 2], 16 * (g // 2 + 1))
                    else:
                        # xcs implies msem (SP gates c2 DMAs on the memset)
                        pe.wait_ge(xcs[g % 2], 16 * (g // 2 + 1))
                    if k >= 8:
                        pe.wait_ge(dvsem, k - 7)  # psum bank k%8 evacuated
                    v = vs[c]
                    kp = 120
                    m = 112  # c2 writes garbage to psum rows 32-111; unread
                    ps = pss[k % 8]
                    for j in range(9):
                        dy, dx = j // 3 - 1, j % 3 - 1
                        jy, py = divmod(dy + 1, 2)
                        jx, px = divmod(dx + 1, 2)
                        rhs = v[
                            0:kp,
                            8 * g + jy : 8 * g + jy + 8,
                            py : py + 1,
                            px : px + 1,
                            jx : jx + 64,
                        ]
                        mm = pe.matmul(
                            ps[0:m, :],
                            wtile[0:kp, j * 128 : j * 128 + m],
                            rhs,
                            start=(j == 0),
                            stop=(j == 8),
                        )
                        if j == 8:
                            mm.then_inc(pesem, 1)

    nc.compile()
    return nc


def _get_module():
    global _MODULE
    if _MODULE is None:
        _MODULE = _build_module()
    return _MODULE


def _band_weights(w5):
    """wc[(zr*8+ic), j*128 + ozr*16 + oc] banded weights, zr = 2*ozr+dz+1."""
    w3 = w5[:, :, 1:4, 1:4, 1:4]  # central 3x3x3 (rest is exactly zero)
    wc = np.zeros((120, 9, 128), np.float32)
    for j in range(9):
        dy, dx = j // 3 - 1, j % 3 - 1
        for dz in (-1, 0, 1):
            blk = w3[:, :, dz + 1, dy + 1, dx + 1].T  # [ic, oc]
            for ozr in range(7):
                zr = 2 * ozr + dz + 1
                wc[zr * 8 : (zr + 1) * 8, j, ozr * 16 : ozr * 16 + 16] = blk
    return np.ascontiguousarray(wc.reshape(120, 9 * 128))


def _shard_core_input(x, b, gz):
    """Per-core input: 3 z-chunks, de-interleaved planes [nz*8, PLANE]."""
    xp = np.zeros((IC, NZS, 130, 130), np.float32)
    z_lo = 32 * gz - 1
    src_lo, src_hi = max(0, z_lo), min(128, z_lo + NZS)
    xp[:, src_lo - z_lo : src_hi - z_lo, 1:129, 1:129] = x[b, :, src_lo:src_hi]
    # de-interleave: y = 2*ye+py-1, x = 2*xe+px-1
    xd = xp.reshape(IC, NZS, 65, 2, 65, 2).transpose(0, 1, 2, 3, 5, 4)
    chunks = []
    for c in range(3):
        base, nz = CHUNK_BASE[c], CHUNK_NZ[c]
        ch = xd[:, base : base + nz].transpose(1, 0, 2, 3, 4, 5)
        chunks.append(np.ascontiguousarray(ch.reshape(nz * 8, PLANE), dtype=BF16))
    return chunks


def kernel(x, weight, bias, psi_local):
    global LAST_RESULT
    from concourse.bass_utils import run_bass_kernel_spmd

    x = np.asarray(x, np.float32)
    weight = np.asarray(weight, np.float32)
    bias = np.asarray(bias, np.float32)
    psi_local = np.asarray(psi_local, np.float32)

    w5 = np.einsum("ogk,kzyx->ogzyx", weight, psi_local).astype(np.float32)
    wc = _band_weights(w5).astype(BF16)

    in_maps = []
    for core in range(N_CORES):
        b, gz = divmod(core, 4)
        c0, c1, c2 = _shard_core_input(x, b, gz)
        in_maps.append({"x0": c0, "x1": c1, "x2": c2, "wc": wc})

    nc = _get_module()
    trace = bool(int(os.environ.get("KERNEL_TRACE", "0")))
    res = run_bass_kernel_spmd(
        nc, in_maps, core_ids=list(range(N_CORES)), trace=trace
    )
    LAST_RESULT = res

    groups = _groups()
    out = np.empty((2, OC, 64, 64, 64), np.float32)
    oc_t = np.empty((16, OC, 64, 64), np.float32)  # [oz, oc, oy, ox] per core
    for core in range(N_CORES):
        b, gz = divmod(core, 4)
        buf = res.results[core]["outf"].astype(np.float32)
        o = 0
        for c, g in groups:
            noz = CHUNK_NOZ[c]
            arr = buf[o : o + 16 * noz].reshape(noz, 16, 8, 64)
            o += 16 * noz
            oc_t[CHUNK_OZ0[c] : CHUNK_OZ0[c] + noz, :, 8 * g : 8 * g + 8] = arr
        out[b, :, 16 * gz : 16 * gz + 16] = oc_t.transpose(1, 0, 2, 3)
    out += bias[None, :, None, None, None]
    return out
